# revision 32
# baseline (speedup 1.0000x reference)
"""MoE layer (top-2 of 8 experts, d_model=1024, d_hidden=512) on 8 trn2 cores.

Token-parallel: each core processes 1024 of the 8192 tokens against all 8
experts. Gating (logits, top-2, softmax) is computed on-device in fp32;
the two expert MLP matmuls run in fp32r (full PE speed). The gate weight is
folded into the combine step as a per-partition scalar multiply-accumulate,
so non-selected experts contribute 0 exactly as in the reference math.

Layout notes:
  - x arrives host-transposed per-shard as xT [D, TC] so both MLP matmuls can
    contract over the partition dimension with weights in native layout.
  - mm1 produces hT [C, tokens] (expert weights stationary), mm2 flips back to
    token-major y [tokens, D] (hT chunks stationary) so the gate is a
    per-partition [128,1] scalar and the output DMAs out in native layout.
"""

import os
import sys

import numpy as np

for _p in ("/opt/trn_rl_repo", "/root/.axon_site/_ro/trn_rl_repo"):
    if _p not in sys.path and os.path.isdir(_p):
        sys.path.append(_p)

P = 128
D_MODEL = 1024
C_HID = 512
N_EXP = 8
TOP_K = 2
N_CORES = 8
T_FULL = 4 * 2048
TC = T_FULL // N_CORES  # tokens per core

KC = D_MODEL // P  # 8 contraction chunks over D
CC = C_HID // P    # 4 contraction chunks over C
TT = TC // P       # 8 token chunks of 128
NT = 512           # moving-dim chunk (tokens) for mm1
DH = 512           # moving-dim chunk (d_model) for mm2

_CACHE = {}

# set by test harness to capture profiling info
TRACE = False
LAST_RESULT = None


def _install_ntff_hook_shim():
    """Register the axon NTFF profile hook if the image's antenv lacks it.

    bass_utils resolves the hook via `antenv.axon_hooks`; when that module is
    absent, tracing silently degrades. The hook implementation itself ships
    with the axon boot package, so wire it up through sys.modules.
    """
    try:
        from antenv.axon_hooks import get_axon_ntff_profile_hook  # noqa: F401
        return  # real module present
    except ImportError:
        pass
    try:
        import types

        if "/root/.axon_site" not in sys.path and os.path.isdir("/root/.axon_site"):
            sys.path.append("/root/.axon_site")
        from trn_agent_boot.trn_boot import _ntff_profile_via_ctypes

        so_path = "/opt/axon/libaxon_pjrt.so"
        if not os.path.exists(so_path):
            return
        hook = _ntff_profile_via_ctypes(so_path)
        mod = types.ModuleType("antenv.axon_hooks")
        mod.get_axon_ntff_profile_hook = lambda: hook
        mod.set_axon_ntff_profile_hook = lambda h: None
        import antenv

        antenv.axon_hooks = mod
        sys.modules["antenv.axon_hooks"] = mod
    except Exception:
        pass


def _split_excess_waits(nc, mybir, maxw=1):
    """This walrus build accepts at most one semaphore wait per instruction.

    Tile emits instructions (notably the kernel-tail drain) with several
    waits; split the extras into preceding single-wait NoOps on the same
    engine — program order makes the chain equivalent.
    """
    for f in nc.m.functions:
        for bb in f.blocks:
            out = []
            changed = False
            for ins in bb.instructions:
                si = ins.sync_info
                waits = list(si.on_wait) if (si is not None and si.on_wait) else []
                if len(waits) > maxw:
                    extra, keep = waits[:-maxw], waits[-maxw:]
                    for ci in range(0, len(extra), maxw):
                        out.append(mybir.InstNoOp(
                            name=f"{ins.name}_ws{ci}",
                            sync_info=mybir.SyncInfo(
                                on_wait=list(extra[ci:ci + maxw]), on_update=[]
                            ),
                            engine=ins.engine,
                            bass_nofuse=True,
                        ))
                    si.on_wait = keep
                    changed = True
                out.append(ins)
            if changed:
                bb.instructions = out


def _build_nc():
    import concourse.bass as bass
    import concourse.mybir as mybir
    import concourse.tile as tile
    from contextlib import ExitStack

    dt = mybir.dt
    f32 = dt.float32
    f32r = dt.float32r
    f16 = dt.float16
    AX = mybir.AxisListType
    OP = mybir.AluOpType
    ACT = mybir.ActivationFunctionType

    nc = bass.Bass("TRN2", debug=False)

    xT = nc.dram_tensor("xT", [D_MODEL, TC], f16, kind="ExternalInput")
    dxT = nc.dram_tensor("dxT", [D_MODEL, TC], f16, kind="ExternalInput")
    wg16 = nc.dram_tensor("wg16", [D_MODEL, N_EXP], f16, kind="ExternalInput")
    dwg = nc.dram_tensor("dwg", [D_MODEL, N_EXP], f16, kind="ExternalInput")
    w1 = nc.dram_tensor("w1", [N_EXP, D_MODEL, C_HID], f16, kind="ExternalInput")
    w2 = nc.dram_tensor("w2", [N_EXP, C_HID, D_MODEL], f16, kind="ExternalInput")
    id8 = nc.dram_tensor("id8", [N_EXP, N_EXP], f32, kind="ExternalInput")
    out = nc.dram_tensor("out", [TC, D_MODEL], f32, kind="ExternalOutput")

    with tile.TileContext(nc) as tc:
        with ExitStack() as ctx:
            cpool = ctx.enter_context(tc.tile_pool(name="cpool", bufs=1))
            wpool = ctx.enter_context(tc.tile_pool(name="wpool", bufs=2))
            hpool = ctx.enter_context(tc.tile_pool(name="hpool", bufs=2))
            gpool = ctx.enter_context(tc.tile_pool(name="gpool", bufs=2))
            psum_mm = ctx.enter_context(tc.tile_pool(name="psum_mm", bufs=4, space="PSUM"))
            psum_sm = ctx.enter_context(tc.tile_pool(name="psum_sm", bufs=2, space="PSUM"))

            xt_sb = cpool.tile([P, KC, TC], f16, name="xt_sb")
            dxt_sb = cpool.tile([P, KC, TC], f16, name="dxt_sb")
            wg_sb = cpool.tile([P, KC, N_EXP], f16, name="wg_sb")
            dwg_sb = cpool.tile([P, KC, N_EXP], f16, name="dwg_sb")
            out_sb = cpool.tile([P, TT, D_MODEL], f32, name="out_sb")
            gate_sb = cpool.tile([P, TT, N_EXP], f32, name="gate_sb")
            id8_sb = cpool.tile([N_EXP, N_EXP], f32, name="id8_sb")
            lgT_sb = cpool.tile([P, TC], f32, name="lgT_sb")

            # DMA order tuned for earliest PE start: expert-0 weights and the
            # fp16 activations feed mm1(e0); the fp32 gating inputs follow in
            # small chunks so logits stream in behind it.
            w1_sb0 = wpool.tile([P, KC, C_HID], f16, name="w1_sb", tag="w1")
            w1r0 = w1[0].rearrange("(kc p) c -> p kc c", p=P)
            nc.sync.dma_start(w1_sb0[:, :, 0:P], w1r0[:, :, 0:P])
            for q2 in range(2):
                sl2 = slice(q2 * 256, (q2 + 1) * 256)
                nc.sync.dma_start(
                    xt_sb[:, :, sl2],
                    xT[:, sl2].rearrange("(kc p) t -> p kc t", p=P))
            for q in range(1, CC):
                nc.sync.dma_start(
                    w1_sb0[:, :, q * P:(q + 1) * P], w1r0[:, :, q * P:(q + 1) * P])
            nc.sync.dma_start(
                xt_sb[:, :, NT:TC],
                xT[:, NT:TC].rearrange("(kc p) t -> p kc t", p=P))
            w2_sb0 = wpool.tile([P, CC, D_MODEL], f16, name="w2_sb", tag="w2")
            nc.sync.dma_start(
                w2_sb0[:], w2[0].rearrange("(cc p) d -> p cc d", p=P))
            nc.sync.dma_start(wg_sb[:], wg16[:].rearrange("(kc p) e -> p kc e", p=P))
            nc.sync.dma_start(dwg_sb[:], dwg[:].rearrange("(kc p) e -> p kc e", p=P))
            nc.sync.dma_start(id8_sb[:], id8[:])
            for th2 in range(2):
                sl = slice(th2 * NT, (th2 + 1) * NT)
                nc.sync.dma_start(
                    dxt_sb[:, :, sl],
                    dxT[:, sl].rearrange("(kc p) t -> p kc t", p=P))

            def emit_mm1(w1_sb):
                ht_sb = hpool.tile([P, CC, TC], f16, name="ht_sb", tag="ht")
                for th in range(TC // NT):
                    for cm in range(CC):
                        ps_h = psum_mm.tile([P, NT], f32, name="ps_h", tag="ps")
                        for kc in range(KC):
                            nc.tensor.matmul(
                                ps_h[:],
                                lhsT=w1_sb[:, kc, cm * P:(cm + 1) * P],
                                rhs=xt_sb[:, kc, th * NT:(th + 1) * NT],
                                start=(kc == 0),
                                stop=(kc == KC - 1),
                            )
                        nc.scalar.activation(
                            ht_sb[:, cm, th * NT:(th + 1) * NT], ps_h[:], ACT.Relu
                        )
                return ht_sb

            def emit_mm2(e, w2_sb, ht_sb):
                for tt in range(TT):
                    for dh in range(D_MODEL // DH):
                        ps_y = psum_mm.tile([P, DH], f32, name="ps_y", tag="ps")
                        for cc in range(CC):
                            nc.tensor.matmul(
                                ps_y[:],
                                lhsT=ht_sb[:, cc, tt * P:(tt + 1) * P],
                                rhs=w2_sb[:, cc, dh * DH:(dh + 1) * DH],
                                start=(cc == 0),
                                stop=(cc == CC - 1),
                            )
                        o_sl = out_sb[:, tt, dh * DH:(dh + 1) * DH]
                        g_col = gate_sb[:, tt, e:e + 1]
                        if e == 0:
                            nc.vector.tensor_single_scalar(
                                o_sl, ps_y[:], g_col, op=OP.mult
                            )
                        else:
                            nc.vector.scalar_tensor_tensor(
                                o_sl, in0=ps_y[:], scalar=g_col, in1=o_sl,
                                op0=OP.mult, op1=OP.add,
                            )

            # expert-0 mm1 first in the PE stream (its inputs land first);
            # its leading token-half runs as N=256 groups to start sooner
            ht_sb0 = hpool.tile([P, CC, TC], f16, name="ht_sb", tag="ht")
            for cm in range(CC):
                for q2 in range(2):
                    ps_h = psum_mm.tile([P, 256], f32, name="ps_h0", tag="ps")
                    for kc in range(KC):
                        nc.tensor.matmul(
                            ps_h[:],
                            lhsT=w1_sb0[:, kc, cm * P:(cm + 1) * P],
                            rhs=xt_sb[:, kc, q2 * 256:(q2 + 1) * 256],
                            start=(kc == 0),
                            stop=(kc == KC - 1),
                        )
                    nc.scalar.activation(
                        ht_sb0[:, cm, q2 * 256:(q2 + 1) * 256], ps_h[:], ACT.Relu)
            for cm in range(CC):
                ps_h = psum_mm.tile([P, NT], f32, name="ps_h", tag="ps")
                for kc in range(KC):
                    nc.tensor.matmul(
                        ps_h[:],
                        lhsT=w1_sb0[:, kc, cm * P:(cm + 1) * P],
                        rhs=xt_sb[:, kc, NT:NT + NT],
                        start=(kc == 0),
                        stop=(kc == KC - 1),
                    )
                nc.scalar.activation(
                    ht_sb0[:, cm, NT:NT + NT], ps_h[:], ACT.Relu)

            # ---- routing: logitsT = Wg.T @ xT (Wg stationary: 8-col weight
            # load is ~free; token dim moves at N=512), then tiny PE
            # transposes back to token-major [128, 8] chunks.
            for th in range(2):
                ps_lt = psum_mm.tile([P, NT], f32, name="ps_lt", tag="ps")
                terms = (
                    [(wg_sb, xt_sb)] * KC + [(wg_sb, dxt_sb)] * KC
                    + [(dwg_sb, xt_sb)] * KC)
                for i, (wt, xt) in enumerate(terms):
                    kc = i % KC
                    nc.tensor.matmul(
                        ps_lt[0:N_EXP, :],
                        lhsT=wt[:, kc, :],
                        rhs=xt[:, kc, th * NT:(th + 1) * NT],
                        start=(i == 0),
                        stop=(i == len(terms) - 1),
                    )
                nc.vector.tensor_copy(
                    lgT_sb[0:N_EXP, th * NT:(th + 1) * NT], ps_lt[0:N_EXP, :])
            for tt in range(TT):
                ps_l = psum_sm.tile([P, N_EXP], f32, name="ps_l", tag="ps_l")
                nc.tensor.transpose(
                    ps_l[:], lgT_sb[0:N_EXP, tt * P:(tt + 1) * P], id8_sb[:])
                lg = gpool.tile([P, N_EXP], f32, name="lg", tag="lg")
                nc.vector.tensor_copy(lg[:], ps_l[:])
                m1 = gpool.tile([P, 1], f32, name="m1", tag="m1")
                nc.vector.reduce_max(m1[:], lg[:], axis=AX.X)
                eq1 = gpool.tile([P, N_EXP], f32, name="eq1", tag="eq1")
                nc.vector.tensor_single_scalar(eq1[:], lg[:], m1[:], op=OP.is_equal)
                msk = gpool.tile([P, N_EXP], f32, name="msk", tag="msk")
                # msk = logits - 1e30 * eq1  (knock out the argmax)
                nc.vector.scalar_tensor_tensor(
                    msk[:], in0=eq1[:], scalar=-1e30, in1=lg[:], op0=OP.mult, op1=OP.add
                )
                m2 = gpool.tile([P, 1], f32, name="m2", tag="m2")
                nc.vector.reduce_max(m2[:], msk[:], axis=AX.X)
                eq2 = gpool.tile([P, N_EXP], f32, name="eq2", tag="eq2")
                nc.vector.tensor_single_scalar(eq2[:], msk[:], m2[:], op=OP.is_equal)
                dlt = gpool.tile([P, 1], f32, name="dlt", tag="dlt")
                nc.vector.tensor_tensor(dlt[:], m2[:], m1[:], op=OP.subtract)
                p2 = gpool.tile([P, 1], f32, name="p2", tag="p2")
                nc.scalar.activation(p2[:], dlt[:], ACT.Sigmoid)
                p1 = gpool.tile([P, 1], f32, name="p1", tag="p1")
                nc.vector.tensor_scalar(
                    p1[:], p2[:], -1.0, 1.0, op0=OP.mult, op1=OP.add
                )
                g1 = gpool.tile([P, N_EXP], f32, name="g1", tag="g1")
                nc.vector.tensor_single_scalar(g1[:], eq1[:], p1[:], op=OP.mult)
                g2 = gpool.tile([P, N_EXP], f32, name="g2", tag="g2")
                nc.vector.tensor_single_scalar(g2[:], eq2[:], p2[:], op=OP.mult)
                nc.vector.tensor_add(gate_sb[:, tt, :], g1[:], g2[:])

            # ---- experts: out[t, :] = sum_e gate[t, e] * relu(x_t @ W1[e]) @ W2[e]
            emit_mm2(0, w2_sb0, ht_sb0)
            for e in range(1, N_EXP):
                w1_sb = wpool.tile([P, KC, C_HID], f16, name="w1_sb", tag="w1")
                nc.sync.dma_start(
                    w1_sb[:], w1[e].rearrange("(kc p) c -> p kc c", p=P)
                )
                w2_sb = wpool.tile([P, CC, D_MODEL], f16, name="w2_sb", tag="w2")
                nc.sync.dma_start(
                    w2_sb[:], w2[e].rearrange("(cc p) d -> p cc d", p=P)
                )
                ht_sb = emit_mm1(w1_sb)
                emit_mm2(e, w2_sb, ht_sb)

            for tt in range(TT):
                nc.sync.dma_start(
                    out[tt * P:(tt + 1) * P, :], out_sb[:, tt, :])

    _split_excess_waits(nc, mybir)
    return nc


def _get_nc():
    if "nc" not in _CACHE:
        _CACHE["nc"] = _build_nc()
    return _CACHE["nc"]


def kernel(**inputs) -> np.ndarray:
    global LAST_RESULT
    x = np.ascontiguousarray(np.asarray(inputs["x"], dtype=np.float32))
    Wg = np.ascontiguousarray(np.asarray(inputs["Wg"], dtype=np.float32))
    W1 = np.ascontiguousarray(np.asarray(inputs["W1"], dtype=np.float32))
    W2 = np.ascontiguousarray(np.asarray(inputs["W2"], dtype=np.float32))

    B, S, D = x.shape
    xf = x.reshape(B * S, D)
    w1h = np.ascontiguousarray(W1.astype(np.float16))
    w2h = np.ascontiguousarray(W2.astype(np.float16))
    wg16c = np.ascontiguousarray(Wg.astype(np.float16))
    dwgc = np.ascontiguousarray((Wg - wg16c.astype(np.float32)).astype(np.float16))
    in_maps = []
    for i in range(N_CORES):
        shard = xf[i * TC:(i + 1) * TC]
        xt = np.ascontiguousarray(shard.T)
        xt16 = np.ascontiguousarray(xt.astype(np.float16))
        in_maps.append({
            "xT": xt16,
            "dxT": np.ascontiguousarray(
                (xt - xt16.astype(np.float32)).astype(np.float16)),
            "wg16": wg16c,
            "dwg": dwgc,
            "id8": np.eye(N_EXP, dtype=np.float32),
            "w1": w1h,
            "w2": w2h,
        })

    from concourse.bass_utils import run_bass_kernel_spmd

    _install_ntff_hook_shim()
    nc = _get_nc()
    res = run_bass_kernel_spmd(
        nc, in_maps, core_ids=list(range(N_CORES)), trace=TRACE
    )
    LAST_RESULT = res
    out = np.concatenate([r["out"] for r in res.results], axis=0)
    return out.reshape(B, S, D)


# revision 33
# speedup vs baseline: 1.0055x; 1.0055x over previous
"""MoE layer (top-2 of 8 experts, d_model=1024, d_hidden=512) on 8 trn2 cores.

Token-parallel: each core processes 1024 of the 8192 tokens against all 8
experts. Gating (logits, top-2, softmax) is computed on-device in fp32;
the two expert MLP matmuls run in fp32r (full PE speed). The gate weight is
folded into the combine step as a per-partition scalar multiply-accumulate,
so non-selected experts contribute 0 exactly as in the reference math.

Layout notes:
  - x arrives host-transposed per-shard as xT [D, TC] so both MLP matmuls can
    contract over the partition dimension with weights in native layout.
  - mm1 produces hT [C, tokens] (expert weights stationary), mm2 flips back to
    token-major y [tokens, D] (hT chunks stationary) so the gate is a
    per-partition [128,1] scalar and the output DMAs out in native layout.
"""

import os
import sys

import numpy as np

for _p in ("/opt/trn_rl_repo", "/root/.axon_site/_ro/trn_rl_repo"):
    if _p not in sys.path and os.path.isdir(_p):
        sys.path.append(_p)

P = 128
D_MODEL = 1024
C_HID = 512
N_EXP = 8
TOP_K = 2
N_CORES = 8
T_FULL = 4 * 2048
TC = T_FULL // N_CORES  # tokens per core

KC = D_MODEL // P  # 8 contraction chunks over D
CC = C_HID // P    # 4 contraction chunks over C
TT = TC // P       # 8 token chunks of 128
NT = 512           # moving-dim chunk (tokens) for mm1
DH = 512           # moving-dim chunk (d_model) for mm2

_CACHE = {}

# set by test harness to capture profiling info
TRACE = False
LAST_RESULT = None


def _install_ntff_hook_shim():
    """Register the axon NTFF profile hook if the image's antenv lacks it.

    bass_utils resolves the hook via `antenv.axon_hooks`; when that module is
    absent, tracing silently degrades. The hook implementation itself ships
    with the axon boot package, so wire it up through sys.modules.
    """
    try:
        from antenv.axon_hooks import get_axon_ntff_profile_hook  # noqa: F401
        return  # real module present
    except ImportError:
        pass
    try:
        import types

        if "/root/.axon_site" not in sys.path and os.path.isdir("/root/.axon_site"):
            sys.path.append("/root/.axon_site")
        from trn_agent_boot.trn_boot import _ntff_profile_via_ctypes

        so_path = "/opt/axon/libaxon_pjrt.so"
        if not os.path.exists(so_path):
            return
        hook = _ntff_profile_via_ctypes(so_path)
        mod = types.ModuleType("antenv.axon_hooks")
        mod.get_axon_ntff_profile_hook = lambda: hook
        mod.set_axon_ntff_profile_hook = lambda h: None
        import antenv

        antenv.axon_hooks = mod
        sys.modules["antenv.axon_hooks"] = mod
    except Exception:
        pass


def _split_excess_waits(nc, mybir, maxw=1):
    """This walrus build accepts at most one semaphore wait per instruction.

    Tile emits instructions (notably the kernel-tail drain) with several
    waits; split the extras into preceding single-wait NoOps on the same
    engine — program order makes the chain equivalent.
    """
    for f in nc.m.functions:
        for bb in f.blocks:
            out = []
            changed = False
            for ins in bb.instructions:
                si = ins.sync_info
                waits = list(si.on_wait) if (si is not None and si.on_wait) else []
                if len(waits) > maxw:
                    extra, keep = waits[:-maxw], waits[-maxw:]
                    for ci in range(0, len(extra), maxw):
                        out.append(mybir.InstNoOp(
                            name=f"{ins.name}_ws{ci}",
                            sync_info=mybir.SyncInfo(
                                on_wait=list(extra[ci:ci + maxw]), on_update=[]
                            ),
                            engine=ins.engine,
                            bass_nofuse=True,
                        ))
                    si.on_wait = keep
                    changed = True
                out.append(ins)
            if changed:
                bb.instructions = out


def _build_nc():
    import concourse.bass as bass
    import concourse.mybir as mybir
    import concourse.tile as tile
    from contextlib import ExitStack

    dt = mybir.dt
    f32 = dt.float32
    f32r = dt.float32r
    f16 = dt.float16
    AX = mybir.AxisListType
    OP = mybir.AluOpType
    ACT = mybir.ActivationFunctionType

    nc = bass.Bass("TRN2", debug=False)

    xT = nc.dram_tensor("xT", [D_MODEL, TC], f16, kind="ExternalInput")
    dxT = nc.dram_tensor("dxT", [D_MODEL, TC], f16, kind="ExternalInput")
    wg16 = nc.dram_tensor("wg16", [D_MODEL, N_EXP], f16, kind="ExternalInput")
    dwg = nc.dram_tensor("dwg", [D_MODEL, N_EXP], f16, kind="ExternalInput")
    w1 = nc.dram_tensor("w1", [N_EXP, D_MODEL, C_HID], f16, kind="ExternalInput")
    w2 = nc.dram_tensor("w2", [N_EXP, C_HID, D_MODEL], f16, kind="ExternalInput")
    id8 = nc.dram_tensor("id8", [N_EXP, N_EXP], f32, kind="ExternalInput")
    out = nc.dram_tensor("out", [TC, D_MODEL], f32, kind="ExternalOutput")

    with tile.TileContext(nc) as tc:
        with ExitStack() as ctx:
            cpool = ctx.enter_context(tc.tile_pool(name="cpool", bufs=1))
            wpool = ctx.enter_context(tc.tile_pool(name="wpool", bufs=2))
            hpool = ctx.enter_context(tc.tile_pool(name="hpool", bufs=2))
            gpool = ctx.enter_context(tc.tile_pool(name="gpool", bufs=2))
            psum_mm = ctx.enter_context(tc.tile_pool(name="psum_mm", bufs=4, space="PSUM"))
            psum_sm = ctx.enter_context(tc.tile_pool(name="psum_sm", bufs=2, space="PSUM"))

            xt_sb = cpool.tile([P, KC, TC], f16, name="xt_sb")
            dxt_sb = cpool.tile([P, KC, TC], f16, name="dxt_sb")
            wg_sb = cpool.tile([P, KC, N_EXP], f16, name="wg_sb")
            dwg_sb = cpool.tile([P, KC, N_EXP], f16, name="dwg_sb")
            out_sb = cpool.tile([P, TT, D_MODEL], f32, name="out_sb")
            gate_sb = cpool.tile([P, TT, N_EXP], f32, name="gate_sb")
            id8_sb = cpool.tile([N_EXP, N_EXP], f32, name="id8_sb")
            lgT_sb = cpool.tile([P, TC], f32, name="lgT_sb")

            # DMA order tuned for earliest PE start: expert-0 weights and the
            # fp16 activations feed mm1(e0); the fp32 gating inputs follow in
            # small chunks so logits stream in behind it.
            w1_sb0 = wpool.tile([P, KC, C_HID], f16, name="w1_sb", tag="w1")
            w1r0 = w1[0].rearrange("(kc p) c -> p kc c", p=P)
            nc.sync.dma_start(w1_sb0[:, :, 0:P], w1r0[:, :, 0:P])
            nc.sync.dma_start(
                xt_sb[:, :, 0:NT],
                xT[:, 0:NT].rearrange("(kc p) t -> p kc t", p=P))
            for q in range(1, CC):
                nc.sync.dma_start(
                    w1_sb0[:, :, q * P:(q + 1) * P], w1r0[:, :, q * P:(q + 1) * P])
            nc.sync.dma_start(
                xt_sb[:, :, NT:TC],
                xT[:, NT:TC].rearrange("(kc p) t -> p kc t", p=P))
            w2_sb0 = wpool.tile([P, CC, D_MODEL], f16, name="w2_sb", tag="w2")
            nc.sync.dma_start(
                w2_sb0[:], w2[0].rearrange("(cc p) d -> p cc d", p=P))
            nc.sync.dma_start(wg_sb[:], wg16[:].rearrange("(kc p) e -> p kc e", p=P))
            nc.sync.dma_start(dwg_sb[:], dwg[:].rearrange("(kc p) e -> p kc e", p=P))
            nc.sync.dma_start(id8_sb[:], id8[:])
            for th2 in range(2):
                sl = slice(th2 * NT, (th2 + 1) * NT)
                nc.sync.dma_start(
                    dxt_sb[:, :, sl],
                    dxT[:, sl].rearrange("(kc p) t -> p kc t", p=P))

            def emit_mm1(w1_sb):
                ht_sb = hpool.tile([P, CC, TC], f16, name="ht_sb", tag="ht")
                for th in range(TC // NT):
                    for cm in range(CC):
                        ps_h = psum_mm.tile([P, NT], f32, name="ps_h", tag="ps")
                        for kc in range(KC):
                            nc.tensor.matmul(
                                ps_h[:],
                                lhsT=w1_sb[:, kc, cm * P:(cm + 1) * P],
                                rhs=xt_sb[:, kc, th * NT:(th + 1) * NT],
                                start=(kc == 0),
                                stop=(kc == KC - 1),
                            )
                        nc.scalar.activation(
                            ht_sb[:, cm, th * NT:(th + 1) * NT], ps_h[:], ACT.Relu
                        )
                return ht_sb

            def emit_mm2(e, w2_sb, ht_sb):
                for tt in range(TT):
                    for dh in range(D_MODEL // DH):
                        ps_y = psum_mm.tile([P, DH], f32, name="ps_y", tag="ps")
                        for cc in range(CC):
                            nc.tensor.matmul(
                                ps_y[:],
                                lhsT=ht_sb[:, cc, tt * P:(tt + 1) * P],
                                rhs=w2_sb[:, cc, dh * DH:(dh + 1) * DH],
                                start=(cc == 0),
                                stop=(cc == CC - 1),
                            )
                        o_sl = out_sb[:, tt, dh * DH:(dh + 1) * DH]
                        g_col = gate_sb[:, tt, e:e + 1]
                        if e == 0:
                            nc.vector.tensor_single_scalar(
                                o_sl, ps_y[:], g_col, op=OP.mult
                            )
                        else:
                            nc.vector.scalar_tensor_tensor(
                                o_sl, in0=ps_y[:], scalar=g_col, in1=o_sl,
                                op0=OP.mult, op1=OP.add,
                            )

            # expert-0 mm1 first in the PE stream (its inputs land first)
            ht_sb0 = emit_mm1(w1_sb0)

            # ---- routing: logitsT = Wg.T @ xT (Wg stationary: 8-col weight
            # load is ~free; token dim moves at N=512), then tiny PE
            # transposes back to token-major [128, 8] chunks.
            for th in range(2):
                ps_lt = psum_mm.tile([P, NT], f32, name="ps_lt", tag="ps")
                terms = (
                    [(wg_sb, xt_sb)] * KC + [(wg_sb, dxt_sb)] * KC
                    + [(dwg_sb, xt_sb)] * KC)
                for i, (wt, xt) in enumerate(terms):
                    kc = i % KC
                    nc.tensor.matmul(
                        ps_lt[0:N_EXP, :],
                        lhsT=wt[:, kc, :],
                        rhs=xt[:, kc, th * NT:(th + 1) * NT],
                        start=(i == 0),
                        stop=(i == len(terms) - 1),
                    )
                nc.vector.tensor_copy(
                    lgT_sb[0:N_EXP, th * NT:(th + 1) * NT], ps_lt[0:N_EXP, :])
            for tt in range(TT):
                ps_l = psum_sm.tile([P, N_EXP], f32, name="ps_l", tag="ps_l")
                nc.tensor.transpose(
                    ps_l[:], lgT_sb[0:N_EXP, tt * P:(tt + 1) * P], id8_sb[:])
                lg = gpool.tile([P, N_EXP], f32, name="lg", tag="lg")
                nc.vector.tensor_copy(lg[:], ps_l[:])
                m1 = gpool.tile([P, 1], f32, name="m1", tag="m1")
                nc.vector.reduce_max(m1[:], lg[:], axis=AX.X)
                eq1 = gpool.tile([P, N_EXP], f32, name="eq1", tag="eq1")
                nc.vector.tensor_single_scalar(eq1[:], lg[:], m1[:], op=OP.is_equal)
                msk = gpool.tile([P, N_EXP], f32, name="msk", tag="msk")
                # msk = logits - 1e30 * eq1  (knock out the argmax)
                nc.vector.scalar_tensor_tensor(
                    msk[:], in0=eq1[:], scalar=-1e30, in1=lg[:], op0=OP.mult, op1=OP.add
                )
                m2 = gpool.tile([P, 1], f32, name="m2", tag="m2")
                nc.vector.reduce_max(m2[:], msk[:], axis=AX.X)
                eq2 = gpool.tile([P, N_EXP], f32, name="eq2", tag="eq2")
                nc.vector.tensor_single_scalar(eq2[:], msk[:], m2[:], op=OP.is_equal)
                dlt = gpool.tile([P, 1], f32, name="dlt", tag="dlt")
                nc.vector.tensor_tensor(dlt[:], m2[:], m1[:], op=OP.subtract)
                p2 = gpool.tile([P, 1], f32, name="p2", tag="p2")
                nc.scalar.activation(p2[:], dlt[:], ACT.Sigmoid)
                p1 = gpool.tile([P, 1], f32, name="p1", tag="p1")
                nc.vector.tensor_scalar(
                    p1[:], p2[:], -1.0, 1.0, op0=OP.mult, op1=OP.add
                )
                g1 = gpool.tile([P, N_EXP], f32, name="g1", tag="g1")
                nc.vector.tensor_single_scalar(g1[:], eq1[:], p1[:], op=OP.mult)
                g2 = gpool.tile([P, N_EXP], f32, name="g2", tag="g2")
                nc.vector.tensor_single_scalar(g2[:], eq2[:], p2[:], op=OP.mult)
                nc.vector.tensor_add(gate_sb[:, tt, :], g1[:], g2[:])

            # ---- experts: out[t, :] = sum_e gate[t, e] * relu(x_t @ W1[e]) @ W2[e]
            emit_mm2(0, w2_sb0, ht_sb0)
            for e in range(1, N_EXP):
                w1_sb = wpool.tile([P, KC, C_HID], f16, name="w1_sb", tag="w1")
                nc.sync.dma_start(
                    w1_sb[:], w1[e].rearrange("(kc p) c -> p kc c", p=P)
                )
                w2_sb = wpool.tile([P, CC, D_MODEL], f16, name="w2_sb", tag="w2")
                nc.sync.dma_start(
                    w2_sb[:], w2[e].rearrange("(cc p) d -> p cc d", p=P)
                )
                ht_sb = emit_mm1(w1_sb)
                emit_mm2(e, w2_sb, ht_sb)

            for tt in range(TT):
                nc.sync.dma_start(
                    out[tt * P:(tt + 1) * P, :], out_sb[:, tt, :])

    _split_excess_waits(nc, mybir)
    return nc


def _get_nc():
    if "nc" not in _CACHE:
        _CACHE["nc"] = _build_nc()
    return _CACHE["nc"]


def kernel(**inputs) -> np.ndarray:
    global LAST_RESULT
    x = np.ascontiguousarray(np.asarray(inputs["x"], dtype=np.float32))
    Wg = np.ascontiguousarray(np.asarray(inputs["Wg"], dtype=np.float32))
    W1 = np.ascontiguousarray(np.asarray(inputs["W1"], dtype=np.float32))
    W2 = np.ascontiguousarray(np.asarray(inputs["W2"], dtype=np.float32))

    B, S, D = x.shape
    xf = x.reshape(B * S, D)
    w1h = np.ascontiguousarray(W1.astype(np.float16))
    w2h = np.ascontiguousarray(W2.astype(np.float16))
    wg16c = np.ascontiguousarray(Wg.astype(np.float16))
    dwgc = np.ascontiguousarray((Wg - wg16c.astype(np.float32)).astype(np.float16))
    in_maps = []
    for i in range(N_CORES):
        shard = xf[i * TC:(i + 1) * TC]
        xt = np.ascontiguousarray(shard.T)
        xt16 = np.ascontiguousarray(xt.astype(np.float16))
        in_maps.append({
            "xT": xt16,
            "dxT": np.ascontiguousarray(
                (xt - xt16.astype(np.float32)).astype(np.float16)),
            "wg16": wg16c,
            "dwg": dwgc,
            "id8": np.eye(N_EXP, dtype=np.float32),
            "w1": w1h,
            "w2": w2h,
        })

    from concourse.bass_utils import run_bass_kernel_spmd

    _install_ntff_hook_shim()
    nc = _get_nc()
    res = run_bass_kernel_spmd(
        nc, in_maps, core_ids=list(range(N_CORES)), trace=TRACE
    )
    LAST_RESULT = res
    out = np.concatenate([r["out"] for r in res.results], axis=0)
    return out.reshape(B, S, D)


# revision 34
# speedup vs baseline: 1.0065x; 1.0010x over previous
"""MoE layer (top-2 of 8 experts, d_model=1024, d_hidden=512) on 8 trn2 cores.

Token-parallel: each core processes 1024 of the 8192 tokens against all 8
experts. Gating (logits, top-2, softmax) is computed on-device in fp32;
the two expert MLP matmuls run in fp32r (full PE speed). The gate weight is
folded into the combine step as a per-partition scalar multiply-accumulate,
so non-selected experts contribute 0 exactly as in the reference math.

Layout notes:
  - x arrives host-transposed per-shard as xT [D, TC] so both MLP matmuls can
    contract over the partition dimension with weights in native layout.
  - mm1 produces hT [C, tokens] (expert weights stationary), mm2 flips back to
    token-major y [tokens, D] (hT chunks stationary) so the gate is a
    per-partition [128,1] scalar and the output DMAs out in native layout.
"""

import os
import sys

import numpy as np

for _p in ("/opt/trn_rl_repo", "/root/.axon_site/_ro/trn_rl_repo"):
    if _p not in sys.path and os.path.isdir(_p):
        sys.path.append(_p)

P = 128
D_MODEL = 1024
C_HID = 512
N_EXP = 8
TOP_K = 2
N_CORES = 8
T_FULL = 4 * 2048
TC = T_FULL // N_CORES  # tokens per core

KC = D_MODEL // P  # 8 contraction chunks over D
CC = C_HID // P    # 4 contraction chunks over C
TT = TC // P       # 8 token chunks of 128
NT = 512           # moving-dim chunk (tokens) for mm1
DH = 512           # moving-dim chunk (d_model) for mm2

_CACHE = {}

# set by test harness to capture profiling info
TRACE = False
LAST_RESULT = None


def _install_ntff_hook_shim():
    """Register the axon NTFF profile hook if the image's antenv lacks it.

    bass_utils resolves the hook via `antenv.axon_hooks`; when that module is
    absent, tracing silently degrades. The hook implementation itself ships
    with the axon boot package, so wire it up through sys.modules.
    """
    try:
        from antenv.axon_hooks import get_axon_ntff_profile_hook  # noqa: F401
        return  # real module present
    except ImportError:
        pass
    try:
        import types

        if "/root/.axon_site" not in sys.path and os.path.isdir("/root/.axon_site"):
            sys.path.append("/root/.axon_site")
        from trn_agent_boot.trn_boot import _ntff_profile_via_ctypes

        so_path = "/opt/axon/libaxon_pjrt.so"
        if not os.path.exists(so_path):
            return
        hook = _ntff_profile_via_ctypes(so_path)
        mod = types.ModuleType("antenv.axon_hooks")
        mod.get_axon_ntff_profile_hook = lambda: hook
        mod.set_axon_ntff_profile_hook = lambda h: None
        import antenv

        antenv.axon_hooks = mod
        sys.modules["antenv.axon_hooks"] = mod
    except Exception:
        pass


def _split_excess_waits(nc, mybir, maxw=1):
    """This walrus build accepts at most one semaphore wait per instruction.

    Tile emits instructions (notably the kernel-tail drain) with several
    waits; split the extras into preceding single-wait NoOps on the same
    engine — program order makes the chain equivalent.
    """
    for f in nc.m.functions:
        for bb in f.blocks:
            out = []
            changed = False
            for ins in bb.instructions:
                si = ins.sync_info
                waits = list(si.on_wait) if (si is not None and si.on_wait) else []
                if len(waits) > maxw:
                    extra, keep = waits[:-maxw], waits[-maxw:]
                    for ci in range(0, len(extra), maxw):
                        out.append(mybir.InstNoOp(
                            name=f"{ins.name}_ws{ci}",
                            sync_info=mybir.SyncInfo(
                                on_wait=list(extra[ci:ci + maxw]), on_update=[]
                            ),
                            engine=ins.engine,
                            bass_nofuse=True,
                        ))
                    si.on_wait = keep
                    changed = True
                out.append(ins)
            if changed:
                bb.instructions = out


def _build_nc():
    import concourse.bass as bass
    import concourse.mybir as mybir
    import concourse.tile as tile
    from contextlib import ExitStack

    dt = mybir.dt
    f32 = dt.float32
    f32r = dt.float32r
    f16 = dt.float16
    AX = mybir.AxisListType
    OP = mybir.AluOpType
    ACT = mybir.ActivationFunctionType

    nc = bass.Bass("TRN2", debug=False)

    xT = nc.dram_tensor("xT", [D_MODEL, TC], f16, kind="ExternalInput")
    dxT = nc.dram_tensor("dxT", [D_MODEL, TC], f16, kind="ExternalInput")
    wg16 = nc.dram_tensor("wg16", [D_MODEL, N_EXP], f16, kind="ExternalInput")
    dwg = nc.dram_tensor("dwg", [D_MODEL, N_EXP], f16, kind="ExternalInput")
    w1 = nc.dram_tensor("w1", [N_EXP, D_MODEL, C_HID], f16, kind="ExternalInput")
    w2 = nc.dram_tensor("w2", [N_EXP, C_HID, D_MODEL], f16, kind="ExternalInput")
    id8 = nc.dram_tensor("id8", [N_EXP, N_EXP], f32, kind="ExternalInput")
    out = nc.dram_tensor("out", [TC, D_MODEL], f32, kind="ExternalOutput")

    with tile.TileContext(nc) as tc:
        with ExitStack() as ctx:
            cpool = ctx.enter_context(tc.tile_pool(name="cpool", bufs=1))
            wpool = ctx.enter_context(tc.tile_pool(name="wpool", bufs=2))
            hpool = ctx.enter_context(tc.tile_pool(name="hpool", bufs=2))
            gpool = ctx.enter_context(tc.tile_pool(name="gpool", bufs=2))
            psum_mm = ctx.enter_context(tc.tile_pool(name="psum_mm", bufs=4, space="PSUM"))
            psum_sm = ctx.enter_context(tc.tile_pool(name="psum_sm", bufs=3, space="PSUM"))

            xt_sb = cpool.tile([P, KC, TC], f16, name="xt_sb")
            dxt_sb = cpool.tile([P, KC, TC], f16, name="dxt_sb")
            wg_sb = cpool.tile([P, KC, N_EXP], f16, name="wg_sb")
            dwg_sb = cpool.tile([P, KC, N_EXP], f16, name="dwg_sb")
            out_sb = cpool.tile([P, TT, D_MODEL], f32, name="out_sb")
            gate_sb = cpool.tile([P, TT, N_EXP], f32, name="gate_sb")
            id8_sb = cpool.tile([N_EXP, N_EXP], f32, name="id8_sb")
            lgT_sb = cpool.tile([P, TC], f32, name="lgT_sb")

            # DMA order tuned for earliest PE start: expert-0 weights and the
            # fp16 activations feed mm1(e0); the fp32 gating inputs follow in
            # small chunks so logits stream in behind it.
            w1_sb0 = wpool.tile([P, KC, C_HID], f16, name="w1_sb", tag="w1")
            w1r0 = w1[0].rearrange("(kc p) c -> p kc c", p=P)
            nc.sync.dma_start(w1_sb0[:, :, 0:P], w1r0[:, :, 0:P])
            nc.sync.dma_start(
                xt_sb[:, :, 0:NT],
                xT[:, 0:NT].rearrange("(kc p) t -> p kc t", p=P))
            for q in range(1, CC):
                nc.sync.dma_start(
                    w1_sb0[:, :, q * P:(q + 1) * P], w1r0[:, :, q * P:(q + 1) * P])
            nc.sync.dma_start(
                xt_sb[:, :, NT:TC],
                xT[:, NT:TC].rearrange("(kc p) t -> p kc t", p=P))
            w2_sb0 = wpool.tile([P, CC, D_MODEL], f16, name="w2_sb", tag="w2")
            nc.sync.dma_start(
                w2_sb0[:], w2[0].rearrange("(cc p) d -> p cc d", p=P))
            nc.sync.dma_start(wg_sb[:], wg16[:].rearrange("(kc p) e -> p kc e", p=P))
            nc.sync.dma_start(dwg_sb[:], dwg[:].rearrange("(kc p) e -> p kc e", p=P))
            nc.sync.dma_start(id8_sb[:], id8[:])
            for th2 in range(2):
                sl = slice(th2 * NT, (th2 + 1) * NT)
                nc.sync.dma_start(
                    dxt_sb[:, :, sl],
                    dxT[:, sl].rearrange("(kc p) t -> p kc t", p=P))

            def emit_mm1(w1_sb):
                ht_sb = hpool.tile([P, CC, TC], f16, name="ht_sb", tag="ht")
                for th in range(TC // NT):
                    for cm in range(CC):
                        ps_h = psum_mm.tile([P, NT], f32, name="ps_h", tag="ps")
                        for kc in range(KC):
                            nc.tensor.matmul(
                                ps_h[:],
                                lhsT=w1_sb[:, kc, cm * P:(cm + 1) * P],
                                rhs=xt_sb[:, kc, th * NT:(th + 1) * NT],
                                start=(kc == 0),
                                stop=(kc == KC - 1),
                            )
                        nc.scalar.activation(
                            ht_sb[:, cm, th * NT:(th + 1) * NT], ps_h[:], ACT.Relu
                        )
                return ht_sb

            def emit_mm2(e, w2_sb, ht_sb):
                for tt in range(TT):
                    for dh in range(D_MODEL // DH):
                        ps_y = psum_mm.tile([P, DH], f32, name="ps_y", tag="ps")
                        for cc in range(CC):
                            nc.tensor.matmul(
                                ps_y[:],
                                lhsT=ht_sb[:, cc, tt * P:(tt + 1) * P],
                                rhs=w2_sb[:, cc, dh * DH:(dh + 1) * DH],
                                start=(cc == 0),
                                stop=(cc == CC - 1),
                            )
                        o_sl = out_sb[:, tt, dh * DH:(dh + 1) * DH]
                        g_col = gate_sb[:, tt, e:e + 1]
                        if e == 0:
                            nc.vector.tensor_single_scalar(
                                o_sl, ps_y[:], g_col, op=OP.mult
                            )
                        else:
                            nc.vector.scalar_tensor_tensor(
                                o_sl, in0=ps_y[:], scalar=g_col, in1=o_sl,
                                op0=OP.mult, op1=OP.add,
                            )

            # expert-0 mm1 first in the PE stream (its inputs land first)
            ht_sb0 = emit_mm1(w1_sb0)

            # ---- routing: logitsT = Wg.T @ xT (Wg stationary: 8-col weight
            # load is ~free; token dim moves at N=512), then tiny PE
            # transposes back to token-major [128, 8] chunks.
            for th in range(2):
                ps_lt = psum_mm.tile([P, NT], f32, name="ps_lt", tag="ps")
                terms = (
                    [(wg_sb, xt_sb)] * KC + [(wg_sb, dxt_sb)] * KC
                    + [(dwg_sb, xt_sb)] * KC)
                for i, (wt, xt) in enumerate(terms):
                    kc = i % KC
                    nc.tensor.matmul(
                        ps_lt[0:N_EXP, :],
                        lhsT=wt[:, kc, :],
                        rhs=xt[:, kc, th * NT:(th + 1) * NT],
                        start=(i == 0),
                        stop=(i == len(terms) - 1),
                    )
                nc.vector.tensor_copy(
                    lgT_sb[0:N_EXP, th * NT:(th + 1) * NT], ps_lt[0:N_EXP, :])
            for tt in range(TT):
                ps_l = psum_sm.tile([P, N_EXP], f32, name="ps_l", tag="ps_l")
                nc.tensor.transpose(
                    ps_l[:], lgT_sb[0:N_EXP, tt * P:(tt + 1) * P], id8_sb[:])
                lg = gpool.tile([P, N_EXP], f32, name="lg", tag="lg")
                nc.vector.tensor_copy(lg[:], ps_l[:])
                m1 = gpool.tile([P, 1], f32, name="m1", tag="m1")
                nc.vector.reduce_max(m1[:], lg[:], axis=AX.X)
                eq1 = gpool.tile([P, N_EXP], f32, name="eq1", tag="eq1")
                nc.vector.tensor_single_scalar(eq1[:], lg[:], m1[:], op=OP.is_equal)
                msk = gpool.tile([P, N_EXP], f32, name="msk", tag="msk")
                # msk = logits - 1e30 * eq1  (knock out the argmax)
                nc.vector.scalar_tensor_tensor(
                    msk[:], in0=eq1[:], scalar=-1e30, in1=lg[:], op0=OP.mult, op1=OP.add
                )
                m2 = gpool.tile([P, 1], f32, name="m2", tag="m2")
                nc.vector.reduce_max(m2[:], msk[:], axis=AX.X)
                eq2 = gpool.tile([P, N_EXP], f32, name="eq2", tag="eq2")
                nc.vector.tensor_single_scalar(eq2[:], msk[:], m2[:], op=OP.is_equal)
                dlt = gpool.tile([P, 1], f32, name="dlt", tag="dlt")
                nc.vector.tensor_tensor(dlt[:], m2[:], m1[:], op=OP.subtract)
                p2 = gpool.tile([P, 1], f32, name="p2", tag="p2")
                nc.scalar.activation(p2[:], dlt[:], ACT.Sigmoid)
                p1 = gpool.tile([P, 1], f32, name="p1", tag="p1")
                nc.vector.tensor_scalar(
                    p1[:], p2[:], -1.0, 1.0, op0=OP.mult, op1=OP.add
                )
                g1 = gpool.tile([P, N_EXP], f32, name="g1", tag="g1")
                nc.vector.tensor_single_scalar(g1[:], eq1[:], p1[:], op=OP.mult)
                g2 = gpool.tile([P, N_EXP], f32, name="g2", tag="g2")
                nc.vector.tensor_single_scalar(g2[:], eq2[:], p2[:], op=OP.mult)
                nc.vector.tensor_add(gate_sb[:, tt, :], g1[:], g2[:])

            # ---- experts: out[t, :] = sum_e gate[t, e] * relu(x_t @ W1[e]) @ W2[e]
            emit_mm2(0, w2_sb0, ht_sb0)
            for e in range(1, N_EXP):
                w1_sb = wpool.tile([P, KC, C_HID], f16, name="w1_sb", tag="w1")
                nc.sync.dma_start(
                    w1_sb[:], w1[e].rearrange("(kc p) c -> p kc c", p=P)
                )
                w2_sb = wpool.tile([P, CC, D_MODEL], f16, name="w2_sb", tag="w2")
                nc.sync.dma_start(
                    w2_sb[:], w2[e].rearrange("(cc p) d -> p cc d", p=P)
                )
                ht_sb = emit_mm1(w1_sb)
                emit_mm2(e, w2_sb, ht_sb)

            for tt in range(TT):
                nc.sync.dma_start(
                    out[tt * P:(tt + 1) * P, :], out_sb[:, tt, :])

    _split_excess_waits(nc, mybir)
    return nc


def _get_nc():
    if "nc" not in _CACHE:
        _CACHE["nc"] = _build_nc()
    return _CACHE["nc"]


def kernel(**inputs) -> np.ndarray:
    global LAST_RESULT
    x = np.ascontiguousarray(np.asarray(inputs["x"], dtype=np.float32))
    Wg = np.ascontiguousarray(np.asarray(inputs["Wg"], dtype=np.float32))
    W1 = np.ascontiguousarray(np.asarray(inputs["W1"], dtype=np.float32))
    W2 = np.ascontiguousarray(np.asarray(inputs["W2"], dtype=np.float32))

    B, S, D = x.shape
    xf = x.reshape(B * S, D)
    w1h = np.ascontiguousarray(W1.astype(np.float16))
    w2h = np.ascontiguousarray(W2.astype(np.float16))
    wg16c = np.ascontiguousarray(Wg.astype(np.float16))
    dwgc = np.ascontiguousarray((Wg - wg16c.astype(np.float32)).astype(np.float16))
    in_maps = []
    for i in range(N_CORES):
        shard = xf[i * TC:(i + 1) * TC]
        xt = np.ascontiguousarray(shard.T)
        xt16 = np.ascontiguousarray(xt.astype(np.float16))
        in_maps.append({
            "xT": xt16,
            "dxT": np.ascontiguousarray(
                (xt - xt16.astype(np.float32)).astype(np.float16)),
            "wg16": wg16c,
            "dwg": dwgc,
            "id8": np.eye(N_EXP, dtype=np.float32),
            "w1": w1h,
            "w2": w2h,
        })

    from concourse.bass_utils import run_bass_kernel_spmd

    _install_ntff_hook_shim()
    nc = _get_nc()
    res = run_bass_kernel_spmd(
        nc, in_maps, core_ids=list(range(N_CORES)), trace=TRACE
    )
    LAST_RESULT = res
    out = np.concatenate([r["out"] for r in res.results], axis=0)
    return out.reshape(B, S, D)
